# revision 7
# baseline (speedup 1.0000x reference)
"""DeepSeek-MoE Trainium2 kernel (8 NeuronCores, expert-parallel).

Strategy
--------
* Routing (sigmoid + grouped top-k, DeepSeek noaux_tc) is replicated on every
  core in fp32 (top-k margins in this regime are ~2e-5, so bf16 routing would
  flip expert selections).
* Dispatch/combine are dense one-hot matmuls built on-device from the routing
  result (no indirect DMA): rank-within-expert comes from an exclusive cumsum
  over tokens realized as a matmul with triangular/ones masks, and the one-hot
  dispatch matrix D[t, c] = (rank[t, e_slot] == c) is built with per-partition
  tensor_scalar(is_equal) against an iota row.
* Expert parallelism: 4 experts per core (load-balanced bin-packing computed
  on the host at call time from the actual routing), per-slot capacities are
  compile-time (multiples of 128 covering the observed loads + margin).
* Expert weights are downcast to bf16 on the host and packed into a single
  flat DRAM stream in exact consumption order, so every weight DMA is one
  fully-contiguous block (128 descriptors of 6-8KB instead of 512 of 2KB).
* Shared experts are sharded over their intermediate dim (352 channels/core).
* Combine is split into two partial buffers: A = slots {0,1} + shared experts
  (ready after the 2nd expert slot), B = slots {2,3}. A's ReduceScatter runs
  hidden behind the remaining expert GEMMs; only B's small RS is in the tail.
  Partials travel as bf16; core r sums rsA+rsB and returns tokens
  [64r, 64r+64) fp32; the host concatenates the 8 slices.
"""

import numpy as np
import ml_dtypes

T, H, E, K, I = 512, 2048, 32, 8, 1408
NG, TKG = 8, 4
RSF = 2.5
NCORES = 8
P = 128
ISH = 2 * I // NCORES  # 352: shared-expert intermediate slice per core
HT = H // P            # 16 h-tiles
TT = T // P            # 4 token tiles
IT = I // P            # 11 i-tiles
GS = E // NG           # 4 experts per group
BIG = 1.0e9

# f-chunking of the 2I=2816 w13 columns: (offset, width) pairs over I
FCH = [(0, 512), (512, 512), (1024, 384)]
KG = 4                          # ko-tiles per w13 DMA chunk
KOG = [(0, 3), (3, 3), (6, 3), (9, 2)]  # w2 ko-groups per DMA chunk
WDN_IW = [128, 128, 96]         # wdn partition-block heights (352 total)

bf16 = ml_dtypes.bfloat16


def _wstream_blocks():
    """Canonical walk of the flat per-core weight stream: (key, nelems).
    Host packs blocks in this order; device slices by the same offsets.
    Block element layout is [partition][row][col] (row-major, contiguous)."""
    for kg in range(HT // 2):
        yield ("wgu", kg), P * 2 * (2 * ISH)
    for j in range(4):
        for fci, (fo, fw) in enumerate(FCH):
            for kg in range(4):
                yield ("w13", j, fci, kg), P * KG * (2 * fw)
        for hh in range(2):
            for gi, (ko0, kn) in enumerate(KOG):
                yield ("w2", j, hh, gi), P * kn * 1024
    for hh in range(2):
        for io in range(3):
            yield ("wdn", hh, io), WDN_IW[io] * 1024


def _wstream_offsets():
    offs, off = {}, 0
    for key, n in _wstream_blocks():
        offs[key] = off
        off += n
    return offs, off


WOFF, WLEN = _wstream_offsets()


# ----------------------------------------------------------------------------
# Host-side routing mirror (only used to pick expert->core assignment and
# compile-time slot capacities; the device re-computes routing exactly).
# ----------------------------------------------------------------------------
def _host_loads(x, gate_w, bias):
    logits = (x.astype(np.float32) @ gate_w.astype(np.float32)).astype(np.float32)
    scores = (1.0 / (1.0 + np.exp(-logits))).astype(np.float32)
    sb = scores + bias[None, :].astype(np.float32)
    g = sb.reshape(T, NG, GS)
    pair = [g[..., i] + g[..., j] for i in range(GS) for j in range(i + 1, GS)]
    grp = np.max(np.stack(pair, -1), -1)
    gmask = np.zeros((T, NG), np.float32)
    gw = grp.copy()
    for _ in range(TKG):
        mx = gw.max(-1, keepdims=True)
        eq = (gw == mx).astype(np.float32)
        gmask += eq
        gw -= eq * BIG
    emask = np.repeat(gmask, GS, axis=1)
    m = sb + (emask * BIG - BIG)
    kmask = np.zeros((T, E), np.float32)
    for _ in range(K):
        mx = m.max(-1, keepdims=True)
        eq = (m == mx).astype(np.float32)
        kmask += eq
        m -= eq * BIG
    return kmask.sum(0)


def _plan_slots(loads, margin=2):
    caps = (np.ceil((loads + margin) / P).astype(int) * P).clip(P, None)
    order = np.argsort(-(caps * 1000 + loads))
    groups = [[] for _ in range(NCORES)]
    gsum = [0] * NCORES
    for e in order:
        cand = [i for i in sorted(range(NCORES), key=lambda i: (gsum[i], len(groups[i])))
                if len(groups[i]) < 4]
        i = cand[0]
        groups[i].append(int(e))
        gsum[i] += caps[e]
    for i in range(NCORES):
        groups[i].sort(key=lambda e: -caps[e])
    slot_caps = [int(max(caps[groups[i][j]] for i in range(NCORES))) for j in range(4)]
    return groups, slot_caps


# ----------------------------------------------------------------------------
# Device program
# ----------------------------------------------------------------------------
def _build_nc(slot_caps, single_core=False):
    import concourse.mybir as mybir
    import concourse.tile as tile
    from concourse import bacc
    from contextlib import ExitStack

    f32 = mybir.dt.float32
    b16 = mybir.dt.bfloat16
    Alu = mybir.AluOpType
    Act = mybir.ActivationFunctionType
    Ax = mybir.AxisListType

    cts = [c // P for c in slot_caps]            # ctiles per slot
    offs = np.cumsum([0] + slot_caps).tolist()   # D column offsets
    DCOLS = offs[-1]
    NCT = sum(cts)                               # total ctiles on this core
    cbase = np.cumsum([0] + cts).tolist()        # global ctile index base per slot
    CAPMAX = max(slot_caps)

    nc = bacc.Bacc("TRN2", target_bir_lowering=False, debug=False,
                   num_devices=1 if single_core else NCORES)

    # ---- I/O ----
    x_d = nc.dram_tensor("x", [T, H], f32, kind="ExternalInput")
    gw_d = nc.dram_tensor("gate_w", [H, E], f32, kind="ExternalInput")
    bias_d = nc.dram_tensor("bias_b", [P, E], f32, kind="ExternalInput")
    wfl_d = nc.dram_tensor("wflat", [WLEN], b16, kind="ExternalInput")
    sel_d = nc.dram_tensor("sel", [E, 4], f32, kind="ExternalInput")
    iota_d = nc.dram_tensor("iota_r", [P, CAPMAX], f32, kind="ExternalInput")
    triu_d = nc.dram_tensor("triu_b", [P, P], b16, kind="ExternalInput")
    ones_d = nc.dram_tensor("ones_b", [P, P], b16, kind="ExternalInput")
    id32_d = nc.dram_tensor("id_f32", [P, P], f32, kind="ExternalInput")
    id16_d = nc.dram_tensor("id_b16", [P, P], b16, kind="ExternalInput")
    out_d = nc.dram_tensor("out_slice",
                           [T, H] if single_core else [T // NCORES, H], f32,
                           kind="ExternalOutput")

    def wap(key, p, r, c):
        # AP of weight-stream block `key` viewed as [p, r, c]
        off = WOFF[key]
        return (wfl_d.ap()[off:off + p * r * c]
                .rearrange("(p r c) -> p r c", p=p, r=r))

    # partial combine outputs: A = slots {0,1} + shared, B = slots {2,3}
    pa_d = nc.dram_tensor("pa", [T, H], b16, kind="Internal")
    pb_d = nc.dram_tensor("pb", [T, H], b16, kind="Internal")
    rsa_d = nc.dram_tensor("rsa", [T // NCORES, H], b16, kind="Internal")
    rsb_d = nc.dram_tensor("rsb", [T // NCORES, H], b16, kind="Internal")

    def cp(i, out, in_):
        # alternate psum/sbuf copies between DVE and ACT to balance engines
        if i % 2 == 0:
            nc.vector.tensor_copy(out=out, in_=in_)
        else:
            nc.scalar.copy(out, in_)

    xr = x_d.ap().rearrange("(tt p) h -> p tt h", p=P)
    gwr = gw_d.ap().rearrange("(ko p) e -> p ko e", p=P)

    with tile.TileContext(nc) as tc, ExitStack() as ctx:
        pc = ctx.enter_context(tc.tile_pool(name="persist", bufs=1))
        xp = ctx.enter_context(tc.tile_pool(name="xstream", bufs=2))
        wp = ctx.enter_context(tc.tile_pool(name="wstream", bufs=3))
        ap_ = ctx.enter_context(tc.tile_pool(name="acts", bufs=1))
        tp_ = ctx.enter_context(tc.tile_pool(name="tmps", bufs=2))
        sp = ctx.enter_context(tc.tile_pool(name="smalls", bufs=2))
        psA = ctx.enter_context(tc.tile_pool(name="psumA", bufs=2, space="PSUM"))
        psB = ctx.enter_context(tc.tile_pool(name="psumB", bufs=1, space="PSUM"))
        op_ = ctx.enter_context(tc.tile_pool(name="ostage", bufs=2))

        def mmw(k, name):
            # three rotating 2-bank wide accumulators
            return psB.tile([P, 1024], f32, tag=f"mmw{k % 3}", name=name)

        # ---- small constant loads ----
        gw_sb = pc.tile([P, HT, E], f32, tag="gw")
        nc.sync.dma_start(gw_sb[:], gwr)
        bias_sb = pc.tile([P, E], f32, tag="bias")
        nc.sync.dma_start(bias_sb[:], bias_d.ap())
        sel_sb = pc.tile([E, 4], f32, tag="sel")
        nc.sync.dma_start(sel_sb[:], sel_d.ap())
        iota_sb = pc.tile([P, CAPMAX], f32, tag="iota")
        nc.sync.dma_start(iota_sb[:], iota_d.ap())
        triu_sb = pc.tile([P, P], b16, tag="triu")
        nc.sync.dma_start(triu_sb[:], triu_d.ap())
        ones_sb = pc.tile([P, P], b16, tag="ones")
        nc.sync.dma_start(ones_sb[:], ones_d.ap())
        id32_sb = pc.tile([P, P], f32, tag="id32")
        nc.sync.dma_start(id32_sb[:], id32_d.ap())
        id16_sb = pc.tile([P, P], b16, tag="id16")
        nc.sync.dma_start(id16_sb[:], id16_d.ap())

        # ---- stream x in (hc, tt) chunks: cast to bf16, x^T (PE), logits ----
        x_bf = pc.tile([P, TT, H], b16, tag="xb")
        xT_bf = pc.tile([P, HT, T], b16, tag="xTb")
        lg_sb = pc.tile([P, TT, E], f32, tag="lg")
        for hc in range(4):
            xf = xp.tile([P, TT, 512], f32, tag="xf")
            for tt in range(TT):
                nc.sync.dma_start(xf[:, tt, :],
                                  xr[:, tt, hc * 512:(hc + 1) * 512])
            cp(hc, x_bf[:, :, hc * 512:(hc + 1) * 512], xf[:])
            xtf = xp.tile([P, 4, T], f32, tag="xtf")  # [hp, ho_local, t]
            for hl in range(4):
                for tt in range(TT):
                    pt = psA.tile([P, P], f32, tag="sm", name="pt_x")
                    nc.tensor.transpose(pt[:], xf[:, tt, hl * P:(hl + 1) * P],
                                        id32_sb[:])
                    cp(tt, xtf[:, hl, tt * P:(tt + 1) * P], pt[:])
                cp(hl, xT_bf[:, hc * 4 + hl, :], xtf[:, hl, :])
            for tt in range(TT):
                pl = psA.tile([P, E], f32, tag="sm", name="pl")
                for hl in range(4):
                    nc.tensor.matmul(pl[:], xtf[:, hl, tt * P:(tt + 1) * P],
                                     gw_sb[:, hc * 4 + hl, :],
                                     start=(hl == 0), stop=(hl == 3))
                if hc == 0:
                    nc.vector.tensor_copy(out=lg_sb[:, tt, :], in_=pl[:])
                else:
                    nc.vector.tensor_tensor(lg_sb[:, tt, :], lg_sb[:, tt, :],
                                            pl[:], Alu.add)

        # ---- routing (fp32, vector/scalar chain; emitted BEFORE the shared
        # expert GEMMs so it runs concurrently with them on DVE/ACT) ----
        scores = pc.tile([P, TT, NG, GS], f32, tag="scores")
        nc.scalar.activation(scores.rearrange("p t g s -> p t (g s)"), lg_sb[:],
                             Act.Sigmoid)
        sbb = pc.tile([P, TT, NG, GS], f32, tag="sbb")
        nc.vector.tensor_tensor(
            sbb[:], scores[:],
            bias_sb.rearrange("p (g s) -> p g s", g=NG)[:, None, :, :]
            .to_broadcast([P, TT, NG, GS]), Alu.add)

        grp = sp.tile([P, TT, NG], f32, tag="grp")
        pw = sp.tile([P, TT, NG], f32, tag="pw")
        first = True
        for i in range(GS):
            for j in range(i + 1, GS):
                dst = grp if first else pw
                nc.vector.tensor_tensor(dst[:], sbb[:, :, :, i], sbb[:, :, :, j],
                                        Alu.add)
                if not first:
                    nc.vector.tensor_tensor(grp[:], grp[:], pw[:], Alu.max)
                first = False

        gmask = sp.tile([P, TT, NG], f32, tag="gmask")
        tmpg = sp.tile([P, TT, NG], f32, tag="tmpg")
        mxg = sp.tile([P, TT], f32, tag="mxg")
        for r in range(TKG):
            nc.vector.reduce_max(mxg[:], grp[:], axis=Ax.X)
            nc.vector.tensor_tensor(tmpg[:], grp[:],
                                    mxg[:, :, None].to_broadcast([P, TT, NG]),
                                    Alu.is_equal)
            if r == 0:
                nc.vector.tensor_copy(out=gmask[:], in_=tmpg[:])
            else:
                nc.vector.tensor_tensor(gmask[:], gmask[:], tmpg[:], Alu.add)
            if r < TKG - 1:
                nc.vector.tensor_scalar(tmpg[:], tmpg[:], BIG, None, Alu.mult)
                nc.vector.tensor_tensor(grp[:], grp[:], tmpg[:], Alu.subtract)

        m_t = pc.tile([P, TT, NG, GS], f32, tag="mt")
        nc.vector.tensor_scalar(m_t[:], gmask[:, :, :, None]
                                .to_broadcast([P, TT, NG, GS]),
                                BIG, -BIG, Alu.mult, Alu.add)
        nc.vector.tensor_tensor(m_t[:], m_t[:], sbb[:], Alu.add)
        m_f = m_t.rearrange("p t g s -> p t (g s)")

        kmask = pc.tile([P, TT, E], f32, tag="kmask")
        tmpk = sp.tile([P, TT, E], f32, tag="tmpk")
        mxk = sp.tile([P, TT], f32, tag="mxk")
        for r in range(K):
            nc.vector.reduce_max(mxk[:], m_f, axis=Ax.X)
            nc.vector.tensor_tensor(tmpk[:], m_f,
                                    mxk[:, :, None].to_broadcast([P, TT, E]),
                                    Alu.is_equal)
            if r == 0:
                nc.vector.tensor_copy(out=kmask[:], in_=tmpk[:])
            else:
                nc.vector.tensor_tensor(kmask[:], kmask[:], tmpk[:], Alu.add)
            if r < K - 1:
                nc.vector.tensor_scalar(tmpk[:], tmpk[:], BIG, None, Alu.mult)
                nc.vector.tensor_tensor(m_f, m_f, tmpk[:], Alu.subtract)

        wsel = sp.tile([P, TT, E], f32, tag="wsel")
        nc.vector.tensor_tensor(wsel[:], kmask[:],
                                scores.rearrange("p t g s -> p t (g s)"),
                                Alu.mult)
        denom = sp.tile([P, TT], f32, tag="denom")
        nc.vector.reduce_sum(denom[:], wsel[:], axis=Ax.X)
        winv = sp.tile([P, TT], f32, tag="winv")
        nc.vector.reciprocal(winv[:], denom[:])
        nc.vector.tensor_scalar(winv[:], winv[:], RSF, None, Alu.mult)
        W_t = pc.tile([P, TT, E], f32, tag="Wt")
        nc.vector.tensor_tensor(W_t[:], wsel[:],
                                winv[:, :, None].to_broadcast([P, TT, E]),
                                Alu.mult)

        count_bf = sp.tile([P, TT, E], b16, tag="countb")
        nc.scalar.copy(count_bf[:], kmask[:])
        namask = sp.tile([P, TT, E], f32, tag="namask")
        nc.vector.tensor_scalar(namask[:], kmask[:], -1.0e6, 1.0e6,
                                Alu.mult, Alu.add)

        # ---- shared expert gate/up (2 token-tile halves, wgu streamed) ----
        act_sh = pc.tile([P, TT, ISH], b16, tag="actsh")
        for th in range(2):
            pshs = [mmw(2 * th + ttl, f"psh{ttl}") for ttl in range(2)]
            for kg in range(HT // 2):
                wguc = wp.tile([P, 2, 2 * ISH], b16, tag="wguc")
                nc.sync.dma_start(wguc[:], wap(("wgu", kg), P, 2, 2 * ISH))
                for kl in range(2):
                    ko = kg * 2 + kl
                    for ttl in range(2):
                        tt = th * 2 + ttl
                        for q0 in range(0, 2 * ISH, 512):
                            qw = min(512, 2 * ISH - q0)
                            nc.tensor.matmul(
                                pshs[ttl][:, q0:q0 + qw],
                                xT_bf[:, ko, tt * P:(tt + 1) * P],
                                wguc[:, kl, q0:q0 + qw],
                                start=(ko == 0), stop=(ko == HT - 1))
            for ttl in range(2):
                tt = th * 2 + ttl
                tmpsh = tp_.tile([P, ISH], b16, tag="tmpsh")
                nc.scalar.activation(tmpsh[:], pshs[ttl][:, :ISH], Act.Silu)
                nc.vector.tensor_tensor(act_sh[:, tt, :], tmpsh[:],
                                        pshs[ttl][:, ISH:2 * ISH], Alu.mult)

        # ---- dispatch build: rank-in-expert via cumsum matmul, one-hot D ----
        # (emitted before the act_sh transposes so its small PE ops interleave
        # with the routing results already computed during the shared GEMMs)
        baseA = pc.tile([P, TT, E], f32, tag="baseA")
        for mt in range(TT):
            pb = psA.tile([P, E], f32, tag="sm", name="pb")
            for kk in range(mt + 1):
                lhs = ones_sb if kk < mt else triu_sb
                nc.tensor.matmul(pb[:], lhs[:], count_bf[:, kk, :],
                                 start=(kk == 0), stop=(kk == mt))
            nc.vector.tensor_tensor(baseA[:, mt, :], pb[:], namask[:, mt, :],
                                    Alu.add)

        # transpose baseA, W -> [E, t]; select this core's 4 experts via sel
        baT = pc.tile([E, TT, P], f32, tag="baT")
        wT = pc.tile([E, TT, P], f32, tag="wT")
        for tt in range(TT):
            pt1 = psA.tile([E, P], f32, tag="sm", name="pt1")
            nc.tensor.transpose(pt1[:], baseA[:, tt, :], id32_sb[:])
            nc.vector.tensor_copy(out=baT[:, tt, :], in_=pt1[:])
            pt2 = psA.tile([E, P], f32, tag="sm", name="pt2")
            nc.tensor.transpose(pt2[:], W_t[:, tt, :], id32_sb[:])
            nc.scalar.copy(wT[:, tt, :], pt2[:])
        bsel = pc.tile([P, TT, 4], f32, tag="bsel")
        wsel4 = pc.tile([P, TT, 4], f32, tag="wsel4")
        for tt in range(TT):
            pb4 = psA.tile([P, 4], f32, tag="sm", name="pb4")
            nc.tensor.matmul(pb4[:], baT[:, tt, :], sel_sb[:], start=True,
                             stop=True)
            nc.vector.tensor_copy(out=bsel[:, tt, :], in_=pb4[:])
            pw4 = psA.tile([P, 4], f32, tag="sm", name="pw4")
            nc.tensor.matmul(pw4[:], wT[:, tt, :], sel_sb[:], start=True,
                             stop=True)
            nc.scalar.copy(wsel4[:, tt, :], pw4[:])

        # dispatch one-hot D (bf16); combine weights Wc built blockwise -> WcT
        D_sb = pc.tile([P, TT, DCOLS], b16, tag="D")
        WcT = pc.tile([P, NCT, T], b16, tag="WcT")
        for tt in range(TT):
            for j in range(4):
                cap = slot_caps[j]
                nc.vector.tensor_scalar(D_sb[:, tt, offs[j]:offs[j] + cap],
                                        iota_sb[:, :cap], bsel[:, tt, j:j + 1],
                                        None, Alu.is_equal)
                wcs = sp.tile([P, 256], f32, tag="wcs")
                nc.vector.tensor_scalar(wcs[:, :cap], iota_sb[:, :cap],
                                        bsel[:, tt, j:j + 1],
                                        wsel4[:, tt, j:j + 1],
                                        Alu.is_equal, Alu.mult)
                for cl in range(cts[j]):
                    ptw = psA.tile([P, P], f32, tag="sm", name="ptw")
                    nc.tensor.transpose(ptw[:], wcs[:, cl * P:(cl + 1) * P],
                                        id32_sb[:])
                    cp(cl + tt, WcT[:, cbase[j] + cl, tt * P:(tt + 1) * P],
                       ptw[:])

        # transpose act_sh -> [i_s, t]
        actShT = pc.tile([P, 3, T], b16, tag="actShT")
        for tt in range(TT):
            for io in range(3):
                iw = min(P, ISH - io * P)
                pt3 = psA.tile([P, P], b16, tag="sm", name="pt3")
                nc.tensor.transpose(pt3[:iw, :],
                                    act_sh[:, tt, io * P:io * P + iw],
                                    id16_sb[:])
                cp(io + tt, actShT[:iw, io, tt * P:(tt + 1) * P], pt3[:iw, :])

        # ---- dispatch matmul: xeT[h, c] = sum_t x[t,h] D[t,c]  (one wide MM) ----
        xeT = pc.tile([P, HT, DCOLS], b16, tag="xeT")
        NDW = (DCOLS + 1023) // 1024
        pctr = 0
        for ko in range(HT):
            for dch in range(NDW):
                cw = min(1024, DCOLS - dch * 1024)
                px = mmw(pctr, "px")
                pctr += 1
                for tt in range(TT):
                    for q0 in range(0, cw, 512):
                        qw = min(512, cw - q0)
                        nc.tensor.matmul(
                            px[:, q0:q0 + qw],
                            x_bf[:, tt, ko * P:(ko + 1) * P],
                            D_sb[:, tt, dch * 1024 + q0:dch * 1024 + q0 + qw],
                            start=(tt == 0), stop=(tt == TT - 1))
                cp(ko + dch, xeT[:, ko, dch * 1024:dch * 1024 + cw],
                   px[:, :cw])

        # ---- routed experts: ctile-streams processed in pairs so the weight
        # DMA demand stays ~uniform (a lone 1-ctile expert would need 2x the
        # per-core HBM bandwidth to keep the PE fed) ----
        streams = [(j, ci) for j in range(4) for ci in range(cts[j])]
        segs = [streams[:cbase[2]], streams[cbase[2]:]]
        ACT_IO = [(0, 4), (4, 8), (8, IT)]   # actT io-tile range ready per fci
        ye_tiles = {}
        for si, seg in enumerate(segs):
            for p0 in range(0, len(seg), 2):
                elems = seg[p0:p0 + 2]
                ne = len(elems)
                same_j = ne == 2 and elems[0][0] == elems[1][0]
                act = ap_.tile([P, 2, I], b16, tag="act", name="act")
                actT = ap_.tile([P, IT, 256], b16, tag="actT", name="actT")
                for fci, (fo, fw) in enumerate(FCH):
                    pgus = [mmw(pctr + i, f"pgu{i}") for i in range(ne)]
                    pctr += ne
                    for kg in range(4):
                        if same_j:
                            wtag = "xf" if (fci * 4 + kg) % 2 == 0 else "xtf"
                            wg0 = xp.tile([P, KG, 2 * fw], b16, tag=wtag,
                                          name="wg")
                            nc.gpsimd.dma_start(
                                wg0[:], wap(("w13", elems[0][0], fci, kg),
                                            P, KG, 2 * fw))
                            wgs = [wg0] * ne
                        else:
                            wgs = []
                            for i, (j_, ci_) in enumerate(elems):
                                wgi = xp.tile([P, KG, 2 * fw], b16,
                                              tag=("xf" if i == 0 else "xtf"),
                                              name="wg")
                                nc.gpsimd.dma_start(
                                    wgi[:], wap(("w13", j_, fci, kg),
                                                P, KG, 2 * fw))
                                wgs.append(wgi)
                        for kl in range(KG):
                            ko = kg * KG + kl
                            for i, (j_, ci_) in enumerate(elems):
                                lhs = xeT[:, ko, offs[j_] + ci_ * P:
                                          offs[j_] + (ci_ + 1) * P]
                                for q0 in range(0, 2 * fw, 512):
                                    qw = min(512, 2 * fw - q0)
                                    nc.tensor.matmul(pgus[i][:, q0:q0 + qw],
                                                     lhs, wgs[i][:, kl,
                                                                 q0:q0 + qw],
                                                     start=(ko == 0),
                                                     stop=(ko == HT - 1))
                    for i in range(ne):
                        tmpa = tp_.tile([P, 512], b16, tag="tmpact")
                        nc.scalar.activation(tmpa[:, :fw], pgus[i][:, :fw],
                                             Act.Silu)
                        nc.vector.tensor_tensor(act[:, i, fo:fo + fw],
                                                tmpa[:, :fw],
                                                pgus[i][:, fw:2 * fw],
                                                Alu.mult)
                    # pipelined act transposes for the PREVIOUS f-chunk (its
                    # silu/mult certainly retired while this chunk ran)
                    if fci >= 1:
                        io0, io1 = ACT_IO[fci - 1]
                        for i in range(ne):
                            for io in range(io0, io1):
                                pt4 = psA.tile([P, P], b16, tag="sm",
                                               name="pt4")
                                nc.tensor.transpose(
                                    pt4[:], act[:, i, io * P:(io + 1) * P],
                                    id16_sb[:])
                                cp(io, actT[:, io, i * P:(i + 1) * P], pt4[:])
                io0, io1 = ACT_IO[2]
                for i in range(ne):
                    for io in range(io0, io1):
                        pt4 = psA.tile([P, P], b16, tag="sm", name="pt4")
                        nc.tensor.transpose(pt4[:],
                                            act[:, i, io * P:(io + 1) * P],
                                            id16_sb[:])
                        cp(io, actT[:, io, i * P:(i + 1) * P], pt4[:])
                # down-proj ye[c, h] in two h-halves, w2 streamed once per half
                yes = []
                for i, (j_, ci_) in enumerate(elems):
                    g = cbase[j_] + ci_
                    t_ = pc.tile([P, H], b16, tag=f"ye{g}", name=f"ye{g}")
                    ye_tiles[g] = t_
                    yes.append(t_)
                for hh in range(2):
                    pys = [mmw(pctr + i, f"py{i}") for i in range(ne)]
                    pctr += ne
                    for gi, (ko0, kn) in enumerate(KOG):
                        if same_j:
                            w2c0 = wp.tile([P, kn, 1024], b16, tag="w2s",
                                           name="w2c")
                            nc.sync.dma_start(
                                w2c0[:], wap(("w2", elems[0][0], hh, gi),
                                             P, kn, 1024))
                            w2cs = [w2c0] * ne
                        else:
                            w2cs = []
                            for i, (j_, ci_) in enumerate(elems):
                                w2ci = wp.tile([P, kn, 1024], b16, tag="w2s",
                                               name="w2c")
                                nc.sync.dma_start(
                                    w2ci[:], wap(("w2", j_, hh, gi),
                                                 P, kn, 1024))
                                w2cs.append(w2ci)
                        for kl in range(kn):
                            ko = ko0 + kl
                            for i in range(ne):
                                for q0 in (0, 512):
                                    nc.tensor.matmul(
                                        pys[i][:, q0:q0 + 512],
                                        actT[:, ko, i * P:(i + 1) * P],
                                        w2cs[i][:, kl, q0:q0 + 512],
                                        start=(ko == 0), stop=(ko == IT - 1))
                    for i in range(ne):
                        cp(i + hh, yes[i][:, hh * 1024:(hh + 1) * 1024],
                           pys[i][:])

            # combine pass for this segment:
            # A (si=0) = slots {0,1} + shared experts, B (si=1) = slots {2,3}
            lo, hi = (0, cbase[2]) if si == 0 else (cbase[2], NCT)
            part_d = pa_d if si == 0 else pb_d
            nreg = 0
            for hh in range(2):
                if si == 0:
                    wdnc = xp.tile([P, 3, 1024], b16, tag="wdnc", name="wdnc")
                    for io in range(3):
                        iw = WDN_IW[io]
                        nc.sync.dma_start(wdnc[:iw, io, :],
                                          wap(("wdn", hh, io), iw, 1, 1024)
                                          .rearrange("p r c -> p (r c)"))
                for tt in range(TT):
                    po = mmw(pctr, f"po{pctr % 3}")
                    pctr += 1
                    ncb = hi - lo
                    for q, cb in enumerate(range(lo, hi)):
                        for q0 in (0, 512):
                            nc.tensor.matmul(
                                po[:, q0:q0 + 512],
                                WcT[:, cb, tt * P:(tt + 1) * P],
                                ye_tiles[cb][:, hh * 1024 + q0:
                                             hh * 1024 + q0 + 512],
                                start=(q == 0),
                                stop=(si == 1 and q == ncb - 1))
                    if si == 0:
                        for io in range(3):
                            iw = WDN_IW[io]
                            for q0 in (0, 512):
                                nc.tensor.matmul(
                                    po[:, q0:q0 + 512],
                                    actShT[:iw, io, tt * P:(tt + 1) * P],
                                    wdnc[:iw, io, q0:q0 + 512],
                                    start=False, stop=(io == 2))
                    stg = op_.tile([P, 1024], b16, tag="ostg")
                    cp(nreg, stg[:, :512], po[:, :512])
                    cp(nreg + 1, stg[:, 512:], po[:, 512:])
                    nc.gpsimd.dma_start(
                        part_d.ap()[tt * P:(tt + 1) * P,
                                    hh * 1024:(hh + 1) * 1024], stg[:])
                    nreg += 1
            # partial complete: one ReduceScatter over the whole [T, H] buffer
            # (A's RS runs hidden behind the slot-2/3 expert GEMMs)
            if not single_core:
                rs_out = rsa_d if si == 0 else rsb_d
                nc.gpsimd.collective_compute(
                    "ReduceScatter", Alu.add,
                    replica_groups=[list(range(NCORES))],
                    ins=[part_d.ap().opt()],
                    outs=[rs_out.ap().opt()],
                )

        # ---- final: out = rsA + rsB (this core's token slice, fp32) ----
        TS = T // NCORES
        if single_core:
            for q in range(4):
                fa = xp.tile([P, 2, H], b16, tag="xf")
                nc.sync.dma_start(fa[:, 0, :],
                                  pa_d.ap()[q * P:(q + 1) * P, :])
                nc.sync.dma_start(fa[:, 1, :],
                                  pb_d.ap()[q * P:(q + 1) * P, :])
                fo_ = xp.tile([P, H], f32, tag="xtf")
                nc.vector.tensor_tensor(fo_[:], fa[:, 0, :], fa[:, 1, :],
                                        Alu.add)
                nc.sync.dma_start(out_d.ap()[q * P:(q + 1) * P, :], fo_[:])
        else:
            fa = xp.tile([TS, 2, H], b16, tag="xf")
            nc.sync.dma_start(fa[:, 0, :], rsa_d.ap())
            nc.sync.dma_start(fa[:, 1, :], rsb_d.ap())
            fo_ = xp.tile([TS, H], f32, tag="xtf")
            nc.vector.tensor_tensor(fo_[:], fa[:, 0, :], fa[:, 1, :], Alu.add)
            nc.sync.dma_start(out_d.ap(), fo_[:])

    nc.compile()
    return nc


_NC_CACHE = {}


def _pack_inputs(x, gate_w, bias, w13, w2, sgu, sdn, groups, slot_caps):
    """Per-core in_maps. All expert/shared weights are packed into one flat
    bf16 stream in exact DMA consumption order (see _wstream_blocks), so each
    device DMA is a single fully-contiguous block."""
    CAPMAX = max(slot_caps)
    iota = np.tile(np.arange(CAPMAX, dtype=np.float32), (P, 1))
    triu = np.triu(np.ones((P, P), np.float32), 1).astype(bf16)
    ones = np.ones((P, P), bf16)
    id32 = np.eye(P, dtype=np.float32)
    id16 = np.eye(P, dtype=np.float32).astype(bf16)
    bias_b = np.tile(bias[None, :], (P, 1)).astype(np.float32)

    in_maps = []
    for core in range(NCORES):
        sel = np.zeros((E, 4), np.float32)
        for jj, e in enumerate(groups[core]):
            sel[e, jj] = 1.0

        wflat = np.empty(WLEN, dtype=bf16)
        # shared gate/up slice for this core: [H, 2*ISH] = gate|up adjacent
        wgu_sh = np.concatenate(
            [sgu[:, core * ISH:(core + 1) * ISH],
             sgu[:, 2 * I + core * ISH: 2 * I + (core + 1) * ISH]],
            axis=1).astype(bf16)                     # [2048, 704]
        for key, n in _wstream_blocks():
            off = WOFF[key]
            if key[0] == "wgu":
                kg = key[1]
                # block [p, kl, f]: partition p within 128, ko = kg*2+kl
                blk = np.stack([wgu_sh[(kg * 2 + kl) * P:(kg * 2 + kl + 1) * P]
                                for kl in range(2)], axis=1)  # [P, 2, 704]
                wflat[off:off + n] = blk.reshape(-1)
            elif key[0] == "w13":
                _, j, fci, kg = key
                fo, fw = FCH[fci]
                w = w13[groups[core][j]]             # [H, 2I] fp32
                cols = np.concatenate([w[:, fo:fo + fw],
                                       w[:, I + fo:I + fo + fw]], axis=1)
                # block [p, kl, f]: ko = kg*KG+kl, partition p in ko block
                blk = np.stack(
                    [cols[(kg * KG + kl) * P:(kg * KG + kl + 1) * P]
                     for kl in range(KG)], axis=1)   # [P, KG, 2fw]
                wflat[off:off + n] = blk.astype(bf16).reshape(-1)
            elif key[0] == "w2":
                _, j, hh, gi = key
                ko0, kn = KOG[gi]
                w = w2[groups[core][j]]              # [I, H] fp32
                blk = np.stack(
                    [w[(ko0 + kl) * P:(ko0 + kl + 1) * P,
                       hh * 1024:(hh + 1) * 1024] for kl in range(kn)],
                    axis=1)                          # [P, kn, 1024]
                wflat[off:off + n] = blk.astype(bf16).reshape(-1)
            else:  # wdn
                _, hh, io = key
                iw = WDN_IW[io]
                blk = sdn[core * ISH + io * P: core * ISH + io * P + iw,
                          hh * 1024:(hh + 1) * 1024]
                wflat[off:off + n] = blk.astype(bf16).reshape(-1)

        in_maps.append({
            "x": x, "gate_w": gate_w, "bias_b": bias_b,
            "wflat": wflat,
            "sel": sel, "iota_r": iota, "triu_b": triu, "ones_b": ones,
            "id_f32": id32, "id_b16": id16,
        })
    return in_maps


def kernel(hidden_states, residual, gate_w, bias, w13, w2, shared_gate_up,
           shared_down):
    from concourse.bass_utils import run_bass_kernel_spmd

    x = np.ascontiguousarray(np.asarray(hidden_states, np.float32))
    gate_w = np.ascontiguousarray(np.asarray(gate_w, np.float32))
    bias = np.asarray(bias, np.float32)
    w13 = np.asarray(w13, np.float32)
    w2 = np.asarray(w2, np.float32)
    sgu = np.asarray(shared_gate_up, np.float32)
    sdn = np.asarray(shared_down, np.float32)

    loads = _host_loads(x, gate_w, bias)
    groups, slot_caps = _plan_slots(loads)

    key = tuple(slot_caps)
    if key not in _NC_CACHE:
        _NC_CACHE[key] = _build_nc(slot_caps)
    nc = _NC_CACHE[key]

    in_maps = _pack_inputs(x, gate_w, bias, w13, w2, sgu, sdn, groups,
                           slot_caps)
    res = run_bass_kernel_spmd(nc, in_maps, core_ids=list(range(NCORES)))
    out = np.concatenate([res.results[c]["out_slice"] for c in range(NCORES)],
                         axis=0)
    return out.astype(np.float32)


# revision 8
# speedup vs baseline: 1.0801x; 1.0801x over previous
"""DeepSeek-MoE Trainium2 kernel (8 NeuronCores, expert-parallel).

Strategy
--------
* Routing (sigmoid + grouped top-k, DeepSeek noaux_tc) is replicated on every
  core in fp32 (top-k margins in this regime are ~2e-5, so bf16 routing would
  flip expert selections).
* Dispatch/combine are dense one-hot matmuls built on-device from the routing
  result (no indirect DMA): rank-within-expert comes from an exclusive cumsum
  over tokens realized as a matmul with triangular/ones masks, and the one-hot
  dispatch matrix D[t, c] = (rank[t, e_slot] == c) is built with per-partition
  tensor_scalar(is_equal) against an iota row.
* Expert parallelism: 4 experts per core (load-balanced bin-packing computed
  on the host at call time from the actual routing), per-slot capacities are
  compile-time (multiples of 128 covering the observed loads + margin).
* Expert weights are downcast to bf16 on the host and packed into a single
  flat DRAM stream in exact consumption order, so every weight DMA is one
  fully-contiguous block (128 descriptors of 6-8KB instead of 512 of 2KB).
* Expert ctile-streams are processed in pairs so the weight-DMA demand stays
  ~uniform (~286 GB/s); a lone 1-ctile expert would need 2x per-core HBM BW.
* Shared experts are sharded over their intermediate dim (352 channels/core).
* Combine is split: A = slots {0,1} + shared (ReduceScatter'd per h-half as
  soon as ready, hidden behind the remaining expert GEMMs), B = slots {2,3}
  (its hh=0 RS overlaps the hh=1 down-projection; only the last ~1MB RS is
  exposed in the tail). Partials travel bf16; core r returns tokens
  [64r, 64r+64) fp32 and the host concatenates the 8 slices.
"""

import numpy as np
import ml_dtypes

T, H, E, K, I = 512, 2048, 32, 8, 1408
NG, TKG = 8, 4
RSF = 2.5
NCORES = 8
P = 128
ISH = 2 * I // NCORES  # 352: shared-expert intermediate slice per core
HISH = ISH // 2        # 176: one column-half of the shared intermediate
HT = H // P            # 16 h-tiles
TT = T // P            # 4 token tiles
IT = I // P            # 11 i-tiles
GS = E // NG           # 4 experts per group
BIG = 1.0e9

# f-chunking of the 2I=2816 w13 columns: (offset, width) pairs over I
FCH = [(0, 512), (512, 512), (1024, 384)]
KG = 4                          # ko-tiles per w13 DMA chunk
KOG = [(0, 3), (3, 3), (6, 3), (9, 2)]  # w2 ko-groups per DMA chunk
WDN_IW = [128, 128, 96]         # wdn partition-block heights (352 total)
ACT_IO = [(0, 4), (4, 8), (8, IT)]  # actT io-tile range completed per fci

bf16 = ml_dtypes.bfloat16


def _wstream_blocks():
    """Canonical walk of the flat per-core weight stream: (key, nelems).
    Host packs blocks in this order; device slices by the same offsets.
    Block element layout is [partition][row][col] (row-major, contiguous)."""
    for u in range(2):
        for kg in range(4):
            yield ("wgu", u, kg), P * 4 * (2 * HISH)
    for j in range(4):
        for fci, (fo, fw) in enumerate(FCH):
            for kg in range(4):
                yield ("w13", j, fci, kg), P * KG * (2 * fw)
        for hh in range(2):
            for gi, (ko0, kn) in enumerate(KOG):
                yield ("w2", j, hh, gi), P * kn * 1024
    for hh in range(2):
        for io in range(3):
            yield ("wdn", hh, io), WDN_IW[io] * 1024


def _wstream_offsets():
    offs, off = {}, 0
    for key, n in _wstream_blocks():
        offs[key] = off
        off += n
    return offs, off


WOFF, WLEN = _wstream_offsets()


# ----------------------------------------------------------------------------
# Host-side routing mirror (only used to pick expert->core assignment and
# compile-time slot capacities; the device re-computes routing exactly).
# ----------------------------------------------------------------------------
def _host_loads(x, gate_w, bias):
    logits = (x.astype(np.float32) @ gate_w.astype(np.float32)).astype(np.float32)
    scores = (1.0 / (1.0 + np.exp(-logits))).astype(np.float32)
    sb = scores + bias[None, :].astype(np.float32)
    g = sb.reshape(T, NG, GS)
    pair = [g[..., i] + g[..., j] for i in range(GS) for j in range(i + 1, GS)]
    grp = np.max(np.stack(pair, -1), -1)
    gmask = np.zeros((T, NG), np.float32)
    gw = grp.copy()
    for _ in range(TKG):
        mx = gw.max(-1, keepdims=True)
        eq = (gw == mx).astype(np.float32)
        gmask += eq
        gw -= eq * BIG
    emask = np.repeat(gmask, GS, axis=1)
    m = sb + (emask * BIG - BIG)
    kmask = np.zeros((T, E), np.float32)
    for _ in range(K):
        mx = m.max(-1, keepdims=True)
        eq = (m == mx).astype(np.float32)
        kmask += eq
        m -= eq * BIG
    return kmask.sum(0)


def _plan_slots(loads, margin=2):
    caps = (np.ceil((loads + margin) / P).astype(int) * P).clip(P, None)
    order = np.argsort(-(caps * 1000 + loads))
    groups = [[] for _ in range(NCORES)]
    gsum = [0] * NCORES
    for e in order:
        cand = [i for i in sorted(range(NCORES), key=lambda i: (gsum[i], len(groups[i])))
                if len(groups[i]) < 4]
        i = cand[0]
        groups[i].append(int(e))
        gsum[i] += caps[e]
    for i in range(NCORES):
        groups[i].sort(key=lambda e: -caps[e])
    slot_caps = [int(max(caps[groups[i][j]] for i in range(NCORES))) for j in range(4)]
    return groups, slot_caps


# ----------------------------------------------------------------------------
# Device program
# ----------------------------------------------------------------------------
def _build_nc(slot_caps, single_core=False):
    import concourse.mybir as mybir
    import concourse.tile as tile
    from concourse import bacc
    from contextlib import ExitStack

    f32 = mybir.dt.float32
    b16 = mybir.dt.bfloat16
    Alu = mybir.AluOpType
    Act = mybir.ActivationFunctionType
    Ax = mybir.AxisListType

    cts = [c // P for c in slot_caps]            # ctiles per slot
    offs = np.cumsum([0] + slot_caps).tolist()   # D column offsets
    DCOLS = offs[-1]
    NCT = sum(cts)                               # total ctiles on this core
    cbase = np.cumsum([0] + cts).tolist()        # global ctile index base per slot
    CAPMAX = max(slot_caps)

    nc = bacc.Bacc("TRN2", target_bir_lowering=False, debug=False,
                   num_devices=1 if single_core else NCORES)

    # ---- I/O ----
    x_d = nc.dram_tensor("x", [T, H], f32, kind="ExternalInput")
    gw_d = nc.dram_tensor("gate_w", [H, E], f32, kind="ExternalInput")
    bias_d = nc.dram_tensor("bias_b", [P, E], f32, kind="ExternalInput")
    wfl_d = nc.dram_tensor("wflat", [WLEN], b16, kind="ExternalInput")
    sel_d = nc.dram_tensor("sel", [E, 4], f32, kind="ExternalInput")
    iota_d = nc.dram_tensor("iota_r", [P, CAPMAX], f32, kind="ExternalInput")
    triu_d = nc.dram_tensor("triu_b", [P, P], b16, kind="ExternalInput")
    ones_d = nc.dram_tensor("ones_b", [P, P], b16, kind="ExternalInput")
    id32_d = nc.dram_tensor("id_f32", [P, P], f32, kind="ExternalInput")
    id16_d = nc.dram_tensor("id_b16", [P, P], b16, kind="ExternalInput")
    out_d = nc.dram_tensor("out_slice",
                           [T, H] if single_core else [T // NCORES, H], f32,
                           kind="ExternalOutput")

    def wap(key, p, r, c):
        # AP of weight-stream block `key` viewed as [p, r, c]
        off = WOFF[key]
        return (wfl_d.ap()[off:off + p * r * c]
                .rearrange("(p r c) -> p r c", p=p, r=r))

    # partial combine outputs, one per h-half:
    # A = slots {0,1} + shared, B = slots {2,3}
    pa_d = [nc.dram_tensor(f"pa{i}", [T, H // 2], b16, kind="Internal")
            for i in range(2)]
    pb_d = [nc.dram_tensor(f"pb{i}", [T, H // 2], b16, kind="Internal")
            for i in range(2)]
    rsa_d = [nc.dram_tensor(f"rsa{i}", [T // NCORES, H // 2], b16,
                            kind="Internal") for i in range(2)]
    rsb_d = [nc.dram_tensor(f"rsb{i}", [T // NCORES, H // 2], b16,
                            kind="Internal") for i in range(2)]

    def cp(i, out, in_):
        # alternate psum/sbuf copies between DVE and ACT to balance engines
        if i % 2 == 0:
            nc.vector.tensor_copy(out=out, in_=in_)
        else:
            nc.scalar.copy(out, in_)

    xr = x_d.ap().rearrange("(tt p) h -> p tt h", p=P)
    gwr = gw_d.ap().rearrange("(ko p) e -> p ko e", p=P)

    with tile.TileContext(nc) as tc, ExitStack() as ctx:
        pc = ctx.enter_context(tc.tile_pool(name="persist", bufs=1))
        xp = ctx.enter_context(tc.tile_pool(name="xstream", bufs=2))
        wp = ctx.enter_context(tc.tile_pool(name="wstream", bufs=3))
        ap_ = ctx.enter_context(tc.tile_pool(name="acts", bufs=1))
        tp_ = ctx.enter_context(tc.tile_pool(name="tmps", bufs=2))
        sp = ctx.enter_context(tc.tile_pool(name="smalls", bufs=2))
        psA = ctx.enter_context(tc.tile_pool(name="psumA", bufs=2, space="PSUM"))
        psB = ctx.enter_context(tc.tile_pool(name="psumB", bufs=1, space="PSUM"))
        op_ = ctx.enter_context(tc.tile_pool(name="ostage", bufs=2))

        def mmw(k, name):
            # three rotating 2-bank wide accumulators
            return psB.tile([P, 1024], f32, tag=f"mmw{k % 3}", name=name)

        # ---- constants on the x critical path first ----
        id32_sb = pc.tile([P, P], f32, tag="id32")
        nc.sync.dma_start(id32_sb[:], id32_d.ap())
        id16_sb = pc.tile([P, P], b16, tag="id16")
        nc.sync.dma_start(id16_sb[:], id16_d.ap())
        gw_sb = pc.tile([P, HT, E], f32, tag="gw")
        nc.sync.dma_start(gw_sb[:], gwr)
        bias_sb = pc.tile([P, E], f32, tag="bias")
        nc.sync.dma_start(bias_sb[:], bias_d.ap())

        # ---- stream x in (hc, tt) chunks: cast to bf16, x^T (PE), logits ----
        x_bf = pc.tile([P, TT, H], b16, tag="xb")
        xT_bf = pc.tile([P, HT, T], b16, tag="xTb")
        lg_sb = pc.tile([P, TT, E], f32, tag="lg")
        for hc in range(4):
            xf = xp.tile([P, TT, 512], f32, tag="xf")
            for tt in range(TT):
                nc.sync.dma_start(xf[:, tt, :],
                                  xr[:, tt, hc * 512:(hc + 1) * 512])
            cp(hc, x_bf[:, :, hc * 512:(hc + 1) * 512], xf[:])
            xtf = xp.tile([P, 4, T], f32, tag="xtf")  # [hp, ho_local, t]
            for hl in range(4):
                for tt in range(TT):
                    pt = psA.tile([P, P], f32, tag="sm", name="pt_x")
                    nc.tensor.transpose(pt[:], xf[:, tt, hl * P:(hl + 1) * P],
                                        id32_sb[:])
                    cp(tt, xtf[:, hl, tt * P:(tt + 1) * P], pt[:])
                cp(hl, xT_bf[:, hc * 4 + hl, :], xtf[:, hl, :])
            for tt in range(TT):
                pl = psA.tile([P, E], f32, tag="sm", name="pl")
                for hl in range(4):
                    nc.tensor.matmul(pl[:], xtf[:, hl, tt * P:(tt + 1) * P],
                                     gw_sb[:, hc * 4 + hl, :],
                                     start=(hl == 0), stop=(hl == 3))
                if hc == 0:
                    nc.vector.tensor_copy(out=lg_sb[:, tt, :], in_=pl[:])
                else:
                    nc.vector.tensor_tensor(lg_sb[:, tt, :], lg_sb[:, tt, :],
                                            pl[:], Alu.add)

        # ---- remaining small constants (needed from the dispatch build on) --
        sel_sb = pc.tile([E, 4], f32, tag="sel")
        nc.sync.dma_start(sel_sb[:], sel_d.ap())
        iota_sb = pc.tile([P, CAPMAX], f32, tag="iota")
        nc.sync.dma_start(iota_sb[:], iota_d.ap())
        triu_sb = pc.tile([P, P], b16, tag="triu")
        nc.sync.dma_start(triu_sb[:], triu_d.ap())
        ones_sb = pc.tile([P, P], b16, tag="ones")
        nc.sync.dma_start(ones_sb[:], ones_d.ap())

        # ---- routing (fp32, vector/scalar chain; emitted BEFORE the shared
        # expert GEMMs so it runs concurrently with them on DVE/ACT) ----
        scores = pc.tile([P, TT, NG, GS], f32, tag="scores")
        nc.scalar.activation(scores.rearrange("p t g s -> p t (g s)"), lg_sb[:],
                             Act.Sigmoid)
        sbb = pc.tile([P, TT, NG, GS], f32, tag="sbb")
        nc.vector.tensor_tensor(
            sbb[:], scores[:],
            bias_sb.rearrange("p (g s) -> p g s", g=NG)[:, None, :, :]
            .to_broadcast([P, TT, NG, GS]), Alu.add)

        grp = sp.tile([P, TT, NG], f32, tag="grp")
        pw = sp.tile([P, TT, NG], f32, tag="pw")
        first = True
        for i in range(GS):
            for j in range(i + 1, GS):
                dst = grp if first else pw
                nc.vector.tensor_tensor(dst[:], sbb[:, :, :, i], sbb[:, :, :, j],
                                        Alu.add)
                if not first:
                    nc.vector.tensor_tensor(grp[:], grp[:], pw[:], Alu.max)
                first = False

        gmask = sp.tile([P, TT, NG], f32, tag="gmask")
        tmpg = sp.tile([P, TT, NG], f32, tag="tmpg")
        mxg = sp.tile([P, TT], f32, tag="mxg")
        for r in range(TKG):
            nc.vector.reduce_max(mxg[:], grp[:], axis=Ax.X)
            nc.vector.tensor_tensor(tmpg[:], grp[:],
                                    mxg[:, :, None].to_broadcast([P, TT, NG]),
                                    Alu.is_equal)
            if r == 0:
                nc.vector.tensor_copy(out=gmask[:], in_=tmpg[:])
            else:
                nc.vector.tensor_tensor(gmask[:], gmask[:], tmpg[:], Alu.add)
            if r < TKG - 1:
                nc.vector.tensor_scalar(tmpg[:], tmpg[:], BIG, None, Alu.mult)
                nc.vector.tensor_tensor(grp[:], grp[:], tmpg[:], Alu.subtract)

        m_t = pc.tile([P, TT, NG, GS], f32, tag="mt")
        nc.vector.tensor_scalar(m_t[:], gmask[:, :, :, None]
                                .to_broadcast([P, TT, NG, GS]),
                                BIG, -BIG, Alu.mult, Alu.add)
        nc.vector.tensor_tensor(m_t[:], m_t[:], sbb[:], Alu.add)
        m_f = m_t.rearrange("p t g s -> p t (g s)")

        kmask = pc.tile([P, TT, E], f32, tag="kmask")
        tmpk = sp.tile([P, TT, E], f32, tag="tmpk")
        mxk = sp.tile([P, TT], f32, tag="mxk")
        for r in range(K):
            nc.vector.reduce_max(mxk[:], m_f, axis=Ax.X)
            nc.vector.tensor_tensor(tmpk[:], m_f,
                                    mxk[:, :, None].to_broadcast([P, TT, E]),
                                    Alu.is_equal)
            if r == 0:
                nc.vector.tensor_copy(out=kmask[:], in_=tmpk[:])
            else:
                nc.vector.tensor_tensor(kmask[:], kmask[:], tmpk[:], Alu.add)
            if r < K - 1:
                nc.vector.tensor_scalar(tmpk[:], tmpk[:], BIG, None, Alu.mult)
                nc.vector.tensor_tensor(m_f, m_f, tmpk[:], Alu.subtract)

        wsel = sp.tile([P, TT, E], f32, tag="wsel")
        nc.vector.tensor_tensor(wsel[:], kmask[:],
                                scores.rearrange("p t g s -> p t (g s)"),
                                Alu.mult)
        denom = sp.tile([P, TT], f32, tag="denom")
        nc.vector.reduce_sum(denom[:], wsel[:], axis=Ax.X)
        winv = sp.tile([P, TT], f32, tag="winv")
        nc.vector.reciprocal(winv[:], denom[:])
        nc.vector.tensor_scalar(winv[:], winv[:], RSF, None, Alu.mult)
        W_t = pc.tile([P, TT, E], f32, tag="Wt")
        nc.vector.tensor_tensor(W_t[:], wsel[:],
                                winv[:, :, None].to_broadcast([P, TT, E]),
                                Alu.mult)

        count_bf = sp.tile([P, TT, E], b16, tag="countb")
        nc.scalar.copy(count_bf[:], kmask[:])
        namask = sp.tile([P, TT, E], f32, tag="namask")
        nc.vector.tensor_scalar(namask[:], kmask[:], -1.0e6, 1.0e6,
                                Alu.mult, Alu.add)

        # ---- shared expert gate/up: one pass over wgu, split into two
        # intermediate column-halves so all 4 token tiles accumulate at once
        # (psum: 2 halves x 4 tt packed into 2x2 wide accumulators) ----
        act_sh = pc.tile([P, TT, ISH], b16, tag="actsh")
        for u in range(2):
            pshs = [mmw(2 * u + pi, f"psh{pi}") for pi in range(2)]
            for kg in range(4):
                wguc = wp.tile([P, 4, 2 * HISH], b16, tag="wguc")
                nc.sync.dma_start(wguc[:], wap(("wgu", u, kg), P, 4, 2 * HISH))
                for kl in range(4):
                    ko = kg * 4 + kl
                    for tt in range(TT):
                        psl = pshs[tt // 2][:, (tt % 2) * 512:
                                            (tt % 2) * 512 + 2 * HISH]
                        nc.tensor.matmul(psl, xT_bf[:, ko, tt * P:(tt + 1) * P],
                                         wguc[:, kl, :],
                                         start=(ko == 0), stop=(ko == HT - 1))
            for tt in range(TT):
                psl = pshs[tt // 2][:, (tt % 2) * 512:
                                    (tt % 2) * 512 + 2 * HISH]
                tmpsh = tp_.tile([P, HISH], b16, tag="tmpsh")
                nc.scalar.activation(tmpsh[:], psl[:, :HISH], Act.Silu)
                nc.vector.tensor_tensor(act_sh[:, tt, u * HISH:(u + 1) * HISH],
                                        tmpsh[:], psl[:, HISH:2 * HISH],
                                        Alu.mult)

        # ---- dispatch build: rank-in-expert via cumsum matmul, one-hot D ----
        baseA = pc.tile([P, TT, E], f32, tag="baseA")
        for mt in range(TT):
            pb = psA.tile([P, E], f32, tag="sm", name="pb")
            for kk in range(mt + 1):
                lhs = ones_sb if kk < mt else triu_sb
                nc.tensor.matmul(pb[:], lhs[:], count_bf[:, kk, :],
                                 start=(kk == 0), stop=(kk == mt))
            nc.vector.tensor_tensor(baseA[:, mt, :], pb[:], namask[:, mt, :],
                                    Alu.add)

        # transpose baseA, W -> [E, t]; select this core's 4 experts via sel
        baT = pc.tile([E, TT, P], f32, tag="baT")
        wT = pc.tile([E, TT, P], f32, tag="wT")
        for tt in range(TT):
            pt1 = psA.tile([E, P], f32, tag="sm", name="pt1")
            nc.tensor.transpose(pt1[:], baseA[:, tt, :], id32_sb[:])
            nc.vector.tensor_copy(out=baT[:, tt, :], in_=pt1[:])
            pt2 = psA.tile([E, P], f32, tag="sm", name="pt2")
            nc.tensor.transpose(pt2[:], W_t[:, tt, :], id32_sb[:])
            nc.scalar.copy(wT[:, tt, :], pt2[:])
        bsel = pc.tile([P, TT, 4], f32, tag="bsel")
        wsel4 = pc.tile([P, TT, 4], f32, tag="wsel4")
        for tt in range(TT):
            pb4 = psA.tile([P, 4], f32, tag="sm", name="pb4")
            nc.tensor.matmul(pb4[:], baT[:, tt, :], sel_sb[:], start=True,
                             stop=True)
            nc.vector.tensor_copy(out=bsel[:, tt, :], in_=pb4[:])
            pw4 = psA.tile([P, 4], f32, tag="sm", name="pw4")
            nc.tensor.matmul(pw4[:], wT[:, tt, :], sel_sb[:], start=True,
                             stop=True)
            nc.scalar.copy(wsel4[:, tt, :], pw4[:])

        # dispatch one-hot D (bf16); combine weights Wc built blockwise -> WcT
        D_sb = pc.tile([P, TT, DCOLS], b16, tag="D")
        WcT = pc.tile([P, NCT, T], b16, tag="WcT")
        for tt in range(TT):
            for j in range(4):
                cap = slot_caps[j]
                nc.vector.tensor_scalar(D_sb[:, tt, offs[j]:offs[j] + cap],
                                        iota_sb[:, :cap], bsel[:, tt, j:j + 1],
                                        None, Alu.is_equal)
                wcs = sp.tile([P, 256], f32, tag="wcs")
                nc.vector.tensor_scalar(wcs[:, :cap], iota_sb[:, :cap],
                                        bsel[:, tt, j:j + 1],
                                        wsel4[:, tt, j:j + 1],
                                        Alu.is_equal, Alu.mult)
                for cl in range(cts[j]):
                    ptw = psA.tile([P, P], f32, tag="sm", name="ptw")
                    nc.tensor.transpose(ptw[:], wcs[:, cl * P:(cl + 1) * P],
                                        id32_sb[:])
                    cp(cl + tt, WcT[:, cbase[j] + cl, tt * P:(tt + 1) * P],
                       ptw[:])

        # transpose act_sh -> [i_s, t]
        actShT = pc.tile([P, 3, T], b16, tag="actShT")
        for tt in range(TT):
            for io in range(3):
                iw = min(P, ISH - io * P)
                pt3 = psA.tile([P, P], b16, tag="sm", name="pt3")
                nc.tensor.transpose(pt3[:iw, :],
                                    act_sh[:, tt, io * P:io * P + iw],
                                    id16_sb[:])
                cp(io + tt, actShT[:iw, io, tt * P:(tt + 1) * P], pt3[:iw, :])

        # ---- dispatch matmul: xeT[h, c] = sum_t x[t,h] D[t,c]  (one wide MM) ----
        xeT = pc.tile([P, HT, DCOLS], b16, tag="xeT")
        NDW = (DCOLS + 1023) // 1024
        pctr = 0
        for ko in range(HT):
            for dch in range(NDW):
                cw = min(1024, DCOLS - dch * 1024)
                px = mmw(pctr, "px")
                pctr += 1
                for tt in range(TT):
                    for q0 in range(0, cw, 512):
                        qw = min(512, cw - q0)
                        nc.tensor.matmul(
                            px[:, q0:q0 + qw],
                            x_bf[:, tt, ko * P:(ko + 1) * P],
                            D_sb[:, tt, dch * 1024 + q0:dch * 1024 + q0 + qw],
                            start=(tt == 0), stop=(tt == TT - 1))
                cp(ko + dch, xeT[:, ko, dch * 1024:dch * 1024 + cw],
                   px[:, :cw])

        # ---- routed experts: ctile-streams processed in pairs ----
        streams = [(j, ci) for j in range(4) for ci in range(cts[j])]
        segs = [streams[:cbase[2]], streams[cbase[2]:]]
        ye_tiles = {}

        def emit_combine(si, hh, nreg0):
            # combine this segment's ctiles (+ shared experts for A) for one
            # h-half, write the bf16 partial, and ReduceScatter it
            nonlocal pctr
            lo, hi = (0, cbase[2]) if si == 0 else (cbase[2], NCT)
            part = (pa_d if si == 0 else pb_d)[hh]
            if si == 0:
                wdnc = xp.tile([P, 3, 1024], b16, tag="wdnc", name="wdnc")
                for io in range(3):
                    iw = WDN_IW[io]
                    nc.sync.dma_start(wdnc[:iw, io, :],
                                      wap(("wdn", hh, io), iw, 1, 1024)
                                      .rearrange("p r c -> p (r c)"))
            for tt in range(TT):
                po = mmw(pctr, f"po{pctr % 3}")
                pctr += 1
                ncb = hi - lo
                for q, cb in enumerate(range(lo, hi)):
                    for q0 in (0, 512):
                        nc.tensor.matmul(
                            po[:, q0:q0 + 512],
                            WcT[:, cb, tt * P:(tt + 1) * P],
                            ye_tiles[cb][:, hh * 1024 + q0:hh * 1024 + q0 + 512],
                            start=(q == 0),
                            stop=(si == 1 and q == ncb - 1))
                if si == 0:
                    for io in range(3):
                        iw = WDN_IW[io]
                        for q0 in (0, 512):
                            nc.tensor.matmul(
                                po[:, q0:q0 + 512],
                                actShT[:iw, io, tt * P:(tt + 1) * P],
                                wdnc[:iw, io, q0:q0 + 512],
                                start=False, stop=(io == 2))
                stg = op_.tile([P, 1024], b16, tag="ostg")
                cp(nreg0 + tt, stg[:, :512], po[:, :512])
                cp(nreg0 + tt + 1, stg[:, 512:], po[:, 512:])
                nc.gpsimd.dma_start(part.ap()[tt * P:(tt + 1) * P, :], stg[:])
            if not single_core:
                rs_out = (rsa_d if si == 0 else rsb_d)[hh]
                nc.gpsimd.collective_compute(
                    "ReduceScatter", Alu.add,
                    replica_groups=[list(range(NCORES))],
                    ins=[part.ap().opt()],
                    outs=[rs_out.ap().opt()],
                )

        for si, seg in enumerate(segs):
            pairs = [seg[p0:p0 + 2] for p0 in range(0, len(seg), 2)]
            for pi, elems in enumerate(pairs):
                ne = len(elems)
                same_j = ne == 2 and elems[0][0] == elems[1][0]
                last = pi == len(pairs) - 1
                act = ap_.tile([P, 2, I], b16, tag="act", name="act")
                actT = ap_.tile([P, IT, 256], b16, tag="actT", name="actT")

                def emit_actT(i, fci):
                    io0, io1 = ACT_IO[fci]
                    for io in range(io0, io1):
                        pt4 = psA.tile([P, P], b16, tag="sm", name="pt4")
                        nc.tensor.transpose(pt4[:],
                                            act[:, i, io * P:(io + 1) * P],
                                            id16_sb[:])
                        cp(io, actT[:, io, i * P:(i + 1) * P], pt4[:])

                for fci, (fo, fw) in enumerate(FCH):
                    pgus = [mmw(pctr + i, f"pgu{i}") for i in range(ne)]
                    pctr += ne
                    for kg in range(4):
                        if same_j:
                            wtag = "xf" if (fci * 4 + kg) % 2 == 0 else "xtf"
                            wg0 = xp.tile([P, KG, 2 * fw], b16, tag=wtag,
                                          name="wg")
                            nc.gpsimd.dma_start(
                                wg0[:], wap(("w13", elems[0][0], fci, kg),
                                            P, KG, 2 * fw))
                            wgs = [wg0] * ne
                        else:
                            wgs = []
                            for i, (j_, ci_) in enumerate(elems):
                                wgi = xp.tile([P, KG, 2 * fw], b16,
                                              tag=("xf" if i == 0 else "xtf"),
                                              name="wg")
                                nc.gpsimd.dma_start(
                                    wgi[:], wap(("w13", j_, fci, kg),
                                                P, KG, 2 * fw))
                                wgs.append(wgi)
                        for kl in range(KG):
                            ko = kg * KG + kl
                            for i, (j_, ci_) in enumerate(elems):
                                lhs = xeT[:, ko, offs[j_] + ci_ * P:
                                          offs[j_] + (ci_ + 1) * P]
                                for q0 in range(0, 2 * fw, 512):
                                    qw = min(512, 2 * fw - q0)
                                    nc.tensor.matmul(pgus[i][:, q0:q0 + qw],
                                                     lhs,
                                                     wgs[i][:, kl, q0:q0 + qw],
                                                     start=(ko == 0),
                                                     stop=(ko == HT - 1))
                    for i in range(ne):
                        tmpa = tp_.tile([P, 512], b16, tag="tmpact")
                        nc.scalar.activation(tmpa[:, :fw], pgus[i][:, :fw],
                                             Act.Silu)
                        nc.vector.tensor_tensor(act[:, i, fo:fo + fw],
                                                tmpa[:, :fw],
                                                pgus[i][:, fw:2 * fw],
                                                Alu.mult)
                    # pipelined act transposes for the PREVIOUS f-chunk (its
                    # silu/mult certainly retired while this chunk ran)
                    if fci >= 1:
                        for i in range(ne):
                            emit_actT(i, fci - 1)
                for i in range(ne):
                    emit_actT(i, 2)

                # down-proj ye[c, h] in two h-halves, w2 streamed once per half
                yes = []
                for i, (j_, ci_) in enumerate(elems):
                    g = cbase[j_] + ci_
                    t_ = pc.tile([P, H], b16, tag=f"ye{g}", name=f"ye{g}")
                    ye_tiles[g] = t_
                    yes.append(t_)
                for hh in range(2):
                    pys = [mmw(pctr + i, f"py{i}") for i in range(ne)]
                    pctr += ne
                    for gi, (ko0, kn) in enumerate(KOG):
                        if same_j:
                            w2c0 = wp.tile([P, kn, 1024], b16, tag="w2s",
                                           name="w2c")
                            nc.sync.dma_start(
                                w2c0[:], wap(("w2", elems[0][0], hh, gi),
                                             P, kn, 1024))
                            w2cs = [w2c0] * ne
                        else:
                            w2cs = []
                            for i, (j_, ci_) in enumerate(elems):
                                w2ci = wp.tile([P, kn, 1024], b16, tag="w2s",
                                               name="w2c")
                                nc.sync.dma_start(
                                    w2ci[:], wap(("w2", j_, hh, gi),
                                                 P, kn, 1024))
                                w2cs.append(w2ci)
                        for kl in range(kn):
                            ko = ko0 + kl
                            for i in range(ne):
                                for q0 in (0, 512):
                                    nc.tensor.matmul(
                                        pys[i][:, q0:q0 + 512],
                                        actT[:, ko, i * P:(i + 1) * P],
                                        w2cs[i][:, kl, q0:q0 + 512],
                                        start=(ko == 0), stop=(ko == IT - 1))
                    for i in range(ne):
                        cp(i + hh, yes[i][:, hh * 1024:(hh + 1) * 1024],
                           pys[i][:])
                    # on the segment's last pair, combine + RS this h-half as
                    # soon as its down-proj retires (hh=0's RS then overlaps
                    # the hh=1 down-proj instead of sitting in the tail)
                    if last:
                        emit_combine(si, hh, 2 * si + hh)

        # ---- final: out = rsA + rsB (this core's token slice, fp32) ----
        TS = T // NCORES
        if single_core:
            for q in range(4):
                fo_ = xp.tile([P, H], f32, tag="xtf")
                for hh in range(2):
                    fa = xp.tile([P, 2, 1024], b16, tag="xf")
                    nc.sync.dma_start(fa[:, 0, :],
                                      pa_d[hh].ap()[q * P:(q + 1) * P, :])
                    nc.sync.dma_start(fa[:, 1, :],
                                      pb_d[hh].ap()[q * P:(q + 1) * P, :])
                    nc.vector.tensor_tensor(fo_[:, hh * 1024:(hh + 1) * 1024],
                                            fa[:, 0, :], fa[:, 1, :], Alu.add)
                nc.sync.dma_start(out_d.ap()[q * P:(q + 1) * P, :], fo_[:])
        else:
            fo_ = xp.tile([TS, H], f32, tag="xtf")
            for hh in range(2):
                fa = xp.tile([TS, 2, 1024], b16, tag="xf")
                nc.sync.dma_start(fa[:, 0, :], rsa_d[hh].ap())
                nc.sync.dma_start(fa[:, 1, :], rsb_d[hh].ap())
                nc.vector.tensor_tensor(fo_[:, hh * 1024:(hh + 1) * 1024],
                                        fa[:, 0, :], fa[:, 1, :], Alu.add)
            nc.sync.dma_start(out_d.ap(), fo_[:])

    nc.compile()
    return nc


_NC_CACHE = {}


def _pack_inputs(x, gate_w, bias, w13, w2, sgu, sdn, groups, slot_caps):
    """Per-core in_maps. All expert/shared weights are packed into one flat
    bf16 stream in exact DMA consumption order (see _wstream_blocks), so each
    device DMA is a single fully-contiguous block."""
    CAPMAX = max(slot_caps)
    iota = np.tile(np.arange(CAPMAX, dtype=np.float32), (P, 1))
    triu = np.triu(np.ones((P, P), np.float32), 1).astype(bf16)
    ones = np.ones((P, P), bf16)
    id32 = np.eye(P, dtype=np.float32)
    id16 = np.eye(P, dtype=np.float32).astype(bf16)
    bias_b = np.tile(bias[None, :], (P, 1)).astype(np.float32)

    in_maps = []
    for core in range(NCORES):
        sel = np.zeros((E, 4), np.float32)
        for jj, e in enumerate(groups[core]):
            sel[e, jj] = 1.0

        wflat = np.empty(WLEN, dtype=bf16)
        # shared gate/up slice for this core: [H, 2*ISH] = gate|up adjacent
        wgu_sh = np.concatenate(
            [sgu[:, core * ISH:(core + 1) * ISH],
             sgu[:, 2 * I + core * ISH: 2 * I + (core + 1) * ISH]],
            axis=1).astype(bf16)                     # [2048, 704]
        for key, n in _wstream_blocks():
            off = WOFF[key]
            if key[0] == "wgu":
                _, u, kg = key
                # column half u: [gate_u | up_u], rows ko = kg*4+kl
                cols = np.concatenate(
                    [wgu_sh[:, u * HISH:(u + 1) * HISH],
                     wgu_sh[:, ISH + u * HISH: ISH + (u + 1) * HISH]], axis=1)
                blk = np.stack([cols[(kg * 4 + kl) * P:(kg * 4 + kl + 1) * P]
                                for kl in range(4)], axis=1)  # [P, 4, 352]
                wflat[off:off + n] = blk.reshape(-1)
            elif key[0] == "w13":
                _, j, fci, kg = key
                fo, fw = FCH[fci]
                w = w13[groups[core][j]]             # [H, 2I] fp32
                cols = np.concatenate([w[:, fo:fo + fw],
                                       w[:, I + fo:I + fo + fw]], axis=1)
                blk = np.stack(
                    [cols[(kg * KG + kl) * P:(kg * KG + kl + 1) * P]
                     for kl in range(KG)], axis=1)   # [P, KG, 2fw]
                wflat[off:off + n] = blk.astype(bf16).reshape(-1)
            elif key[0] == "w2":
                _, j, hh, gi = key
                ko0, kn = KOG[gi]
                w = w2[groups[core][j]]              # [I, H] fp32
                blk = np.stack(
                    [w[(ko0 + kl) * P:(ko0 + kl + 1) * P,
                       hh * 1024:(hh + 1) * 1024] for kl in range(kn)],
                    axis=1)                          # [P, kn, 1024]
                wflat[off:off + n] = blk.astype(bf16).reshape(-1)
            else:  # wdn
                _, hh, io = key
                iw = WDN_IW[io]
                blk = sdn[core * ISH + io * P: core * ISH + io * P + iw,
                          hh * 1024:(hh + 1) * 1024]
                wflat[off:off + n] = blk.astype(bf16).reshape(-1)

        in_maps.append({
            "x": x, "gate_w": gate_w, "bias_b": bias_b,
            "wflat": wflat,
            "sel": sel, "iota_r": iota, "triu_b": triu, "ones_b": ones,
            "id_f32": id32, "id_b16": id16,
        })
    return in_maps


def kernel(hidden_states, residual, gate_w, bias, w13, w2, shared_gate_up,
           shared_down):
    from concourse.bass_utils import run_bass_kernel_spmd

    x = np.ascontiguousarray(np.asarray(hidden_states, np.float32))
    gate_w = np.ascontiguousarray(np.asarray(gate_w, np.float32))
    bias = np.asarray(bias, np.float32)
    w13 = np.asarray(w13, np.float32)
    w2 = np.asarray(w2, np.float32)
    sgu = np.asarray(shared_gate_up, np.float32)
    sdn = np.asarray(shared_down, np.float32)

    loads = _host_loads(x, gate_w, bias)
    groups, slot_caps = _plan_slots(loads)

    key = tuple(slot_caps)
    if key not in _NC_CACHE:
        _NC_CACHE[key] = _build_nc(slot_caps)
    nc = _NC_CACHE[key]

    in_maps = _pack_inputs(x, gate_w, bias, w13, w2, sgu, sdn, groups,
                           slot_caps)
    res = run_bass_kernel_spmd(nc, in_maps, core_ids=list(range(NCORES)))
    out = np.concatenate([res.results[c]["out_slice"] for c in range(NCORES)],
                         axis=0)
    return out.astype(np.float32)
